# revision 25
# baseline (speedup 1.0000x reference)
"""Memristive fully-connected layer on 8 Trainium2 NeuronCores.

The reference's differential conductance pair collapses algebraically:
g_pos - g_neg = g_eff = k_cond * weights, and the final rescale divides
K_V * k_cond back out, so the module computes exactly y = x @ w + b.

Strategy: data-parallel over the batch. Each core computes a
(1024 x 4096) @ (4096 x 4096) + b GEMM slice with float32r matmuls
(full-rate fp32 path on the PE array). The x shard is pre-transposed on
host so stationary-operand tiles are contiguous; the whole xT shard
(16.8 MB) stays resident in SBUF and w streams from HBM exactly once
per core. Bias is broadcast across partitions once and added on PSUM
eviction by the vector engine.

Per core: 8 n-blocks of 512 columns; the contraction runs in 8 k-blocks
of 4 k-tiles, sweeping all 8 output-row tiles per k-block, so each PSUM
bank's final matmul sits ~10 us ahead of the next block's first use and
evictions never stall the PE. DMAs are batched (4 k-tiles of w or xT
per transfer, 2 output tiles per store) to respect the 8 hardware DGE
queues, with w on the SP queue and xT/outputs on the Activation queue.
A short burst of throwaway matmuls during the initial DMA fill lifts
the PE's HAM clock gate before real work arrives.
"""

import numpy as np

import concourse.bass as bass  # noqa: F401  (registers engine classes)
import concourse.mybir as mybir
from concourse import bacc, tile
from concourse.bass_utils import run_bass_kernel_spmd

dt = mybir.dt

BATCH, N_IN, N_OUT = 8192, 4096, 4096
NCORES = 8
MB = BATCH // NCORES          # 1024 batch rows per core
P = 128
KT = N_IN // P                # 32 contraction tiles
MT = MB // P                  # 8 output-row tiles per core
NBLK = 512                    # matmul free dim (one PSUM bank)
NB = N_OUT // NBLK            # 8 output-column blocks
KB = 4                        # k-tiles per k-block (per w DMA)
NKB = KT // KB                # 8 k-blocks
WARMUP_MM = 8

_cache = {}


def _build():
    nc = bacc.Bacc("TRN2", target_bir_lowering=False, debug=False)
    xT = nc.dram_tensor("xT", [N_IN, MB], dt.float32r, kind="ExternalInput")
    w = nc.dram_tensor("w", [N_IN, N_OUT], dt.float32r, kind="ExternalInput")
    b = nc.dram_tensor("b", [1, N_OUT], dt.float32, kind="ExternalInput")
    y = nc.dram_tensor("y", [MB, N_OUT], dt.float32, kind="ExternalOutput")

    xT_r = xT.rearrange("(kt p) m -> p kt m", p=P)    # [128, 32, 1024]
    w_r = w.rearrange("(kt p) n -> p kt n", p=P)      # [128, 32, 4096]
    y_r = y.rearrange("(mt p) n -> p mt n", p=P)      # [128, 8, 4096]

    with tile.TileContext(nc) as tc:
        with (
            tc.tile_pool(name="xtp", bufs=1) as xtp,
            tc.tile_pool(name="wp", bufs=5) as wp,
            tc.tile_pool(name="bp", bufs=1) as bp,
            tc.tile_pool(name="op", bufs=3) as op,
            tc.tile_pool(name="ps", bufs=1, space="PSUM") as ps,
        ):
            # w k-block DMA, 4 k-tiles per transfer on the SP queue.
            # Returns the block as a list of per-k-tile [128, 512] views.
            def w_dma(nb, kb):
                wt = wp.tile([P, KB, NBLK], dt.float32r, name="wt")
                nc.sync.dma_start(
                    wt[:],
                    w_r[:, kb * KB:(kb + 1) * KB, nb * NBLK:(nb + 1) * NBLK],
                )
                return [wt[:, kk, :] for kk in range(KB)]

            xts = xtp.tile([P, KT, MB], dt.float32r, name="xts")

            def xt_dma(kb):
                nc.scalar.dma_start(
                    xts[:, kb * KB:(kb + 1) * KB, :],
                    xT_r[:, kb * KB:(kb + 1) * KB, :],
                )

            # HAM warmup: throwaway matmuls on a zeroed tile while the
            # first DMAs are in flight, so real matmuls start at 2.4 GHz.
            warm = bp.tile([P, 256], dt.float32, name="warm")
            nc.gpsimd.memset(warm[:], 0.0)
            wpsums = [
                ps.tile([P, NBLK], dt.float32, name=f"ps{i}") for i in range(MT)
            ]
            for i in range(WARMUP_MM):
                nc.tensor.matmul(
                    wpsums[i % MT][:, :256], warm[:, :P], warm[:],
                    start=True, stop=True,
                )

            # Startup DMAs in consumption order: the 8 hardware DGE queues
            # are assigned round-robin in emission order and each is FIFO,
            # so a soon-needed transfer must not sit behind a later-needed
            # one at a queue head. The first k-block's transfers are split
            # per k-tile so the first matmul's data lands in ~3 us instead
            # of waiting on multi-MB blocks.
            wts0 = [None] * NKB
            # k=0 transfers are tiny so the very first matmul can start
            # ~3 us in; k=1..3 follow as one block each.
            wt00 = bp.tile([P, 1, NBLK], dt.float32r, name="wt00")
            nc.sync.dma_start(wt00[:], w_r[:, 0:1, 0:NBLK])
            nc.scalar.dma_start(xts[:, 0:1, :], xT_r[:, 0:1, :])
            wt03 = wp.tile([P, KB, NBLK], dt.float32r, name="wt")
            nc.sync.dma_start(wt03[:, :KB - 1, :], w_r[:, 1:KB, 0:NBLK])
            nc.scalar.dma_start(xts[:, 1:KB, :], xT_r[:, 1:KB, :])
            wts0[0] = [wt00[:, 0, :]] + [wt03[:, kk, :] for kk in range(KB - 1)]
            wts0[1] = w_dma(0, 1)
            wts0[2] = w_dma(0, 2)
            xt_dma(1)
            wts0[3] = w_dma(0, 3)
            xt_dma(2)
            wts0[4] = w_dma(0, 4)
            xt_dma(3)
            wts0[5] = w_dma(0, 5)
            xt_dma(4)
            wts0[6] = w_dma(0, 6)
            xt_dma(5)
            wts0[7] = w_dma(0, 7)
            for kb in range(6, NKB):
                xt_dma(kb)

            # Bias: DMA the row into partition 0 of bias_sb, then broadcast
            # in place. Emitted after the warmup/startup DMAs — it rides the
            # slow gpsimd queue and is only needed at the first eviction
            # (~95 us in).
            bias_sb = bp.tile([P, N_OUT], dt.float32, name="bias_sb")
            nc.scalar.dma_start(bias_sb[0:1, :], b[:, :])
            nc.gpsimd.partition_broadcast(bias_sb[:], bias_sb[0:1, :])

            for nb in range(NB):
                psums = [
                    ps.tile([P, NBLK], dt.float32, name=f"ps{m}")
                    for m in range(MT)
                ]
                ot = None
                for kb in range(NKB):
                    if nb == 0:
                        wts = wts0[kb]
                    else:
                        wts = w_dma(nb, kb)
                    for m in range(MT):
                        for kk in range(KB):
                            k = kb * KB + kk
                            nc.tensor.matmul(
                                psums[m][:],
                                xts[:, k, m * P:(m + 1) * P],
                                wts[kk],
                                start=(k == 0),
                                stop=(k == KT - 1),
                            )
                        if kb == NKB - 1:
                            if m % 2 == 0:
                                ot = op.tile([P, 2, NBLK], dt.float32, name="ot")
                            nc.vector.tensor_add(
                                ot[:, m % 2, :],
                                psums[m][:],
                                bias_sb[:, nb * NBLK:(nb + 1) * NBLK],
                            )
                            if m % 2 == 1:
                                nc.scalar.dma_start(
                                    y_r[:, m - 1:m + 1, nb * NBLK:(nb + 1) * NBLK],
                                    ot[:],
                                )
    nc.compile()
    return nc


def kernel(x, w, b, _trace=False, _trace_kwargs=None):
    if "nc" not in _cache:
        _cache["nc"] = _build()
    nc = _cache["nc"]

    b2 = np.ascontiguousarray(np.asarray(b, dtype=np.float32).reshape(1, N_OUT))
    w2 = np.ascontiguousarray(np.asarray(w, dtype=np.float32))
    in_maps = []
    for c in range(NCORES):
        xs = np.ascontiguousarray(x[c * MB:(c + 1) * MB].T.astype(np.float32))
        in_maps.append({"xT": xs, "w": w2, "b": b2})

    res = run_bass_kernel_spmd(
        nc,
        in_maps,
        core_ids=list(range(NCORES)),
        trace=_trace,
        **(_trace_kwargs or {}),
    )
    out = np.concatenate([res.results[c]["y"] for c in range(NCORES)], axis=0)
    if _trace:
        return out, res
    return out


# revision 27
# speedup vs baseline: 1.0066x; 1.0066x over previous
"""Memristive fully-connected layer on 8 Trainium2 NeuronCores.

The reference's differential conductance pair collapses algebraically:
g_pos - g_neg = g_eff = k_cond * weights, and the final rescale divides
K_V * k_cond back out, so the module computes exactly y = x @ w + b.

Strategy: data-parallel over the batch. Each core computes a
(1024 x 4096) @ (4096 x 4096) + b GEMM slice with float32r matmuls
(full-rate fp32 path on the PE array). The x shard is pre-transposed on
host so stationary-operand tiles are contiguous; the whole xT shard
(16.8 MB) stays resident in SBUF and w streams from HBM exactly once
per core. Bias is broadcast across partitions once and added on PSUM
eviction by the vector engine.

Per core: 8 n-blocks of 512 columns; the contraction runs in 8 k-blocks
of 4 k-tiles, sweeping all 8 output-row tiles per k-block, so each PSUM
bank's final matmul sits ~10 us ahead of the next block's first use and
evictions never stall the PE. DMAs are batched (4 k-tiles of w or xT
per transfer, 2 output tiles per store) to respect the 8 hardware DGE
queues, with w on the SP queue and xT/outputs on the Activation queue.
A short burst of throwaway matmuls during the initial DMA fill lifts
the PE's HAM clock gate before real work arrives.
"""

import numpy as np

import concourse.bass as bass  # noqa: F401  (registers engine classes)
import concourse.mybir as mybir
from concourse import bacc, tile
from concourse.bass_utils import run_bass_kernel_spmd

dt = mybir.dt

BATCH, N_IN, N_OUT = 8192, 4096, 4096
NCORES = 8
MB = BATCH // NCORES          # 1024 batch rows per core
P = 128
KT = N_IN // P                # 32 contraction tiles
MT = MB // P                  # 8 output-row tiles per core
NBLK = 512                    # matmul free dim (one PSUM bank)
NB = N_OUT // NBLK            # 8 output-column blocks
KB = 4                        # k-tiles per k-block (per w DMA)
NKB = KT // KB                # 8 k-blocks
WARMUP_MM = 64

_cache = {}


def _build():
    nc = bacc.Bacc("TRN2", target_bir_lowering=False, debug=False)
    xT = nc.dram_tensor("xT", [N_IN, MB], dt.float32r, kind="ExternalInput")
    w = nc.dram_tensor("w", [N_IN, N_OUT], dt.float32r, kind="ExternalInput")
    b = nc.dram_tensor("b", [1, N_OUT], dt.float32, kind="ExternalInput")
    y = nc.dram_tensor("y", [MB, N_OUT], dt.float32, kind="ExternalOutput")

    xT_r = xT.rearrange("(kt p) m -> p kt m", p=P)    # [128, 32, 1024]
    w_r = w.rearrange("(kt p) n -> p kt n", p=P)      # [128, 32, 4096]
    y_r = y.rearrange("(mt p) n -> p mt n", p=P)      # [128, 8, 4096]

    with tile.TileContext(nc) as tc:
        with (
            tc.tile_pool(name="xtp", bufs=1) as xtp,
            tc.tile_pool(name="wp", bufs=5) as wp,
            tc.tile_pool(name="bp", bufs=1) as bp,
            tc.tile_pool(name="op", bufs=3) as op,
            tc.tile_pool(name="ps", bufs=1, space="PSUM") as ps,
        ):
            # w k-block DMA, 4 k-tiles per transfer on the SP queue.
            # Returns the block as a list of per-k-tile [128, 512] views.
            def w_dma(nb, kb):
                wt = wp.tile([P, KB, NBLK], dt.float32r, name="wt")
                nc.sync.dma_start(
                    wt[:],
                    w_r[:, kb * KB:(kb + 1) * KB, nb * NBLK:(nb + 1) * NBLK],
                )
                return [wt[:, kk, :] for kk in range(KB)]

            xts = xtp.tile([P, KT, MB], dt.float32r, name="xts")

            def xt_dma(kb):
                nc.scalar.dma_start(
                    xts[:, kb * KB:(kb + 1) * KB, :],
                    xT_r[:, kb * KB:(kb + 1) * KB, :],
                )

            # HAM warmup: throwaway matmuls on a zeroed tile while the
            # first DMAs are in flight, so real matmuls start at 2.4 GHz.
            warm = bp.tile([P, P], dt.float32, name="warm")
            nc.gpsimd.memset(warm[:], 0.0)
            wpsums = [
                ps.tile([P, NBLK], dt.float32, name=f"ps{i}") for i in range(MT)
            ]
            for i in range(WARMUP_MM):
                nc.tensor.matmul(
                    wpsums[i % MT][:, :P], warm[:], warm[:],
                    start=True, stop=True,
                )

            # Startup DMAs in consumption order: the 8 hardware DGE queues
            # are assigned round-robin in emission order and each is FIFO,
            # so a soon-needed transfer must not sit behind a later-needed
            # one at a queue head. The first k-block's transfers are split
            # per k-tile so the first matmul's data lands in ~3 us instead
            # of waiting on multi-MB blocks.
            wts0 = [None] * NKB
            # k=0 transfers are tiny so the very first matmul can start
            # ~3 us in; k=1..3 follow as one block each.
            wt00 = bp.tile([P, 1, NBLK], dt.float32r, name="wt00")
            nc.sync.dma_start(wt00[:], w_r[:, 0:1, 0:NBLK])
            nc.scalar.dma_start(xts[:, 0:1, :], xT_r[:, 0:1, :])
            wt03 = wp.tile([P, KB, NBLK], dt.float32r, name="wt")
            nc.sync.dma_start(wt03[:, :KB - 1, :], w_r[:, 1:KB, 0:NBLK])
            nc.scalar.dma_start(xts[:, 1:KB, :], xT_r[:, 1:KB, :])
            wts0[0] = [wt00[:, 0, :]] + [wt03[:, kk, :] for kk in range(KB - 1)]
            wts0[1] = w_dma(0, 1)
            wts0[2] = w_dma(0, 2)
            xt_dma(1)
            wts0[3] = w_dma(0, 3)
            xt_dma(2)
            wts0[4] = w_dma(0, 4)
            xt_dma(3)
            wts0[5] = w_dma(0, 5)
            xt_dma(4)
            wts0[6] = w_dma(0, 6)
            xt_dma(5)
            wts0[7] = w_dma(0, 7)
            for kb in range(6, NKB):
                xt_dma(kb)

            # Bias: DMA the row into partition 0 of bias_sb, then broadcast
            # in place. Emitted after the warmup/startup DMAs — it rides the
            # slow gpsimd queue and is only needed at the first eviction
            # (~95 us in).
            bias_sb = bp.tile([P, N_OUT], dt.float32, name="bias_sb")
            nc.scalar.dma_start(bias_sb[0:1, :], b[:, :])
            nc.gpsimd.partition_broadcast(bias_sb[:], bias_sb[0:1, :])

            for nb in range(NB):
                psums = [
                    ps.tile([P, NBLK], dt.float32, name=f"ps{m}")
                    for m in range(MT)
                ]
                ot = None
                for kb in range(NKB):
                    if nb == 0:
                        wts = wts0[kb]
                    else:
                        wts = w_dma(nb, kb)
                    for m in range(MT):
                        for kk in range(KB):
                            k = kb * KB + kk
                            nc.tensor.matmul(
                                psums[m][:],
                                xts[:, k, m * P:(m + 1) * P],
                                wts[kk],
                                start=(k == 0),
                                stop=(k == KT - 1),
                            )
                        if kb == NKB - 1:
                            if m % 2 == 0:
                                ot = op.tile([P, 2, NBLK], dt.float32, name="ot")
                            nc.vector.tensor_add(
                                ot[:, m % 2, :],
                                psums[m][:],
                                bias_sb[:, nb * NBLK:(nb + 1) * NBLK],
                            )
                            if m % 2 == 1:
                                nc.scalar.dma_start(
                                    y_r[:, m - 1:m + 1, nb * NBLK:(nb + 1) * NBLK],
                                    ot[:],
                                )
    nc.compile()
    return nc


def kernel(x, w, b, _trace=False, _trace_kwargs=None):
    if "nc" not in _cache:
        _cache["nc"] = _build()
    nc = _cache["nc"]

    b2 = np.ascontiguousarray(np.asarray(b, dtype=np.float32).reshape(1, N_OUT))
    w2 = np.ascontiguousarray(np.asarray(w, dtype=np.float32))
    in_maps = []
    for c in range(NCORES):
        xs = np.ascontiguousarray(x[c * MB:(c + 1) * MB].T.astype(np.float32))
        in_maps.append({"xT": xs, "w": w2, "b": b2})

    res = run_bass_kernel_spmd(
        nc,
        in_maps,
        core_ids=list(range(NCORES)),
        trace=_trace,
        **(_trace_kwargs or {}),
    )
    out = np.concatenate([res.results[c]["y"] for c in range(NCORES)], axis=0)
    if _trace:
        return out, res
    return out


# revision 31
# speedup vs baseline: 1.0226x; 1.0159x over previous
"""Memristive fully-connected layer on 8 Trainium2 NeuronCores.

The reference's differential conductance pair collapses algebraically:
g_pos - g_neg = g_eff = k_cond * weights, and the final rescale divides
K_V * k_cond back out, so the module computes exactly y = x @ w + b.

Strategy: data-parallel over the batch. Each core computes a
(1024 x 4096) @ (4096 x 4096) + b GEMM slice with float32r matmuls
(full-rate fp32 path on the PE array). The x shard is pre-transposed on
host so stationary-operand tiles are contiguous; the whole xT shard
(16.8 MB) stays resident in SBUF and w streams from HBM exactly once
per core. Bias is broadcast across partitions once and added on PSUM
eviction by the vector engine.

Per core: 8 n-blocks of 512 columns; the contraction runs in 8 k-blocks
of 4 k-tiles, sweeping all 8 output-row tiles per k-block, so each PSUM
bank's final matmul sits ~10 us ahead of the next block's first use and
evictions never stall the PE. DMAs are batched (4 k-tiles of w or xT
per transfer, 2 output tiles per store) to respect the 8 hardware DGE
queues, with w on the SP queue and xT/outputs on the Activation queue.
A short burst of throwaway matmuls during the initial DMA fill lifts
the PE's HAM clock gate before real work arrives.
"""

import numpy as np

import concourse.bass as bass  # noqa: F401  (registers engine classes)
import concourse.mybir as mybir
from concourse import bacc, tile
from concourse.bass_utils import run_bass_kernel_spmd

dt = mybir.dt

BATCH, N_IN, N_OUT = 8192, 4096, 4096
NCORES = 8
MB = BATCH // NCORES          # 1024 batch rows per core
P = 128
KT = N_IN // P                # 32 contraction tiles
MT = MB // P                  # 8 output-row tiles per core
NBLK = 512                    # matmul free dim (one PSUM bank)
NB = N_OUT // NBLK            # 8 output-column blocks
KB = 4                        # k-tiles per k-block (per w DMA)
NKB = KT // KB                # 8 k-blocks
WARMUP_MM = 10

_cache = {}


def _build():
    nc = bacc.Bacc("TRN2", target_bir_lowering=False, debug=False)
    xT = nc.dram_tensor("xT", [N_IN, MB], dt.float32r, kind="ExternalInput")
    w = nc.dram_tensor("w", [N_IN, N_OUT], dt.float32r, kind="ExternalInput")
    b = nc.dram_tensor("b", [1, N_OUT], dt.float32, kind="ExternalInput")
    y = nc.dram_tensor("y", [MB, N_OUT], dt.float32, kind="ExternalOutput")

    xT_r = xT.rearrange("(kt p) m -> p kt m", p=P)    # [128, 32, 1024]
    w_r = w.rearrange("(kt p) n -> p kt n", p=P)      # [128, 32, 4096]
    y_r = y.rearrange("(mt p) n -> p mt n", p=P)      # [128, 8, 4096]

    with tile.TileContext(nc) as tc:
        with (
            tc.tile_pool(name="xtp", bufs=1) as xtp,
            tc.tile_pool(name="wp", bufs=4) as wp,
            tc.tile_pool(name="bp", bufs=1) as bp,
            tc.tile_pool(name="op", bufs=3) as op,
            tc.tile_pool(name="ps", bufs=1, space="PSUM") as ps,
        ):
            # w k-block DMA, 4 k-tiles per transfer on the SP queue.
            # Returns the block as a list of per-k-tile [128, 512] views.
            def w_dma(nb, kb):
                wt = wp.tile([P, KB, NBLK], dt.float32r, name="wt")
                nc.sync.dma_start(
                    wt[:],
                    w_r[:, kb * KB:(kb + 1) * KB, nb * NBLK:(nb + 1) * NBLK],
                )
                return [wt[:, kk, :] for kk in range(KB)]

            xts = xtp.tile([P, KT, MB], dt.float32r, name="xts")

            def xt_dma(kb):
                nc.scalar.dma_start(
                    xts[:, kb * KB:(kb + 1) * KB, :],
                    xT_r[:, kb * KB:(kb + 1) * KB, :],
                )

            # HAM warmup: throwaway matmuls on a zeroed tile while the
            # first DMAs are in flight, so real matmuls start at 2.4 GHz.
            warm = bp.tile([P, 256], dt.float32, name="warm")
            nc.gpsimd.memset(warm[:], 0.0)
            wpsums = [
                ps.tile([P, NBLK], dt.float32, name=f"ps{i}") for i in range(MT)
            ]
            for i in range(WARMUP_MM):
                nc.tensor.matmul(
                    wpsums[i % MT][:, :256], warm[:, :P], warm[:],
                    start=True, stop=True,
                )

            # Startup DMAs in consumption order: the 8 hardware DGE queues
            # are assigned round-robin in emission order and each is FIFO,
            # so a soon-needed transfer must not sit behind a later-needed
            # one at a queue head. The first k-block's transfers are split
            # per k-tile so the first matmul's data lands in ~3 us instead
            # of waiting on multi-MB blocks.
            wts0 = [None] * NKB
            wts0[0] = w_dma(0, 0)
            xt_dma(0)
            wts0[1] = w_dma(0, 1)
            wts0[2] = w_dma(0, 2)
            xt_dma(1)
            wts0[3] = w_dma(0, 3)
            xt_dma(2)
            wts0[4] = w_dma(0, 4)
            xt_dma(3)
            wts0[5] = w_dma(0, 5)
            xt_dma(4)
            wts0[6] = w_dma(0, 6)
            xt_dma(5)
            wts0[7] = w_dma(0, 7)
            for kb in range(6, NKB):
                xt_dma(kb)

            # Bias: DMA the row into partition 0 of bias_sb, then broadcast
            # in place. Emitted after the warmup/startup DMAs — it rides the
            # slow gpsimd queue and is only needed at the first eviction
            # (~95 us in).
            bias_sb = bp.tile([P, N_OUT], dt.float32, name="bias_sb")
            nc.scalar.dma_start(bias_sb[0:1, :], b[:, :])
            nc.gpsimd.partition_broadcast(bias_sb[:], bias_sb[0:1, :])

            for nb in range(NB):
                psums = [
                    ps.tile([P, NBLK], dt.float32, name=f"ps{m}")
                    for m in range(MT)
                ]
                ot = None
                for kb in range(NKB):
                    if nb == 0:
                        wts = wts0[kb]
                    else:
                        wts = w_dma(nb, kb)
                    for m in range(MT):
                        for kk in range(KB):
                            k = kb * KB + kk
                            nc.tensor.matmul(
                                psums[m][:],
                                xts[:, k, m * P:(m + 1) * P],
                                wts[kk],
                                start=(k == 0),
                                stop=(k == KT - 1),
                            )
                        if kb == NKB - 1:
                            if m % 2 == 0:
                                ot = op.tile([P, 2, NBLK], dt.float32, name="ot")
                            nc.vector.tensor_add(
                                ot[:, m % 2, :],
                                psums[m][:],
                                bias_sb[:, nb * NBLK:(nb + 1) * NBLK],
                            )
                            if m % 2 == 1:
                                nc.scalar.dma_start(
                                    y_r[:, m - 1:m + 1, nb * NBLK:(nb + 1) * NBLK],
                                    ot[:],
                                )
    nc.compile()
    return nc


def kernel(x, w, b, _trace=False, _trace_kwargs=None):
    if "nc" not in _cache:
        _cache["nc"] = _build()
    nc = _cache["nc"]

    b2 = np.ascontiguousarray(np.asarray(b, dtype=np.float32).reshape(1, N_OUT))
    w2 = np.ascontiguousarray(np.asarray(w, dtype=np.float32))
    in_maps = []
    for c in range(NCORES):
        xs = np.ascontiguousarray(x[c * MB:(c + 1) * MB].T.astype(np.float32))
        in_maps.append({"xT": xs, "w": w2, "b": b2})

    res = run_bass_kernel_spmd(
        nc,
        in_maps,
        core_ids=list(range(NCORES)),
        trace=_trace,
        **(_trace_kwargs or {}),
    )
    out = np.concatenate([res.results[c]["y"] for c in range(NCORES)], axis=0)
    if _trace:
        return out, res
    return out


# revision 32
# speedup vs baseline: 1.0414x; 1.0184x over previous
"""Memristive fully-connected layer on 8 Trainium2 NeuronCores.

The reference's differential conductance pair collapses algebraically:
g_pos - g_neg = g_eff = k_cond * weights, and the final rescale divides
K_V * k_cond back out, so the module computes exactly y = x @ w + b.

Strategy: data-parallel over the batch. Each core computes a
(1024 x 4096) @ (4096 x 4096) + b GEMM slice with float32r matmuls
(full-rate fp32 path on the PE array). The x shard is pre-transposed on
host so stationary-operand tiles are contiguous; the whole xT shard
(16.8 MB) stays resident in SBUF and w streams from HBM exactly once
per core. Bias is broadcast across partitions once and added on PSUM
eviction by the vector engine.

Per core: 8 n-blocks of 512 columns; the contraction runs in 8 k-blocks
of 4 k-tiles, sweeping all 8 output-row tiles per k-block, so each PSUM
bank's final matmul sits ~10 us ahead of the next block's first use and
evictions never stall the PE. DMAs are batched (4 k-tiles of w or xT
per transfer, 2 output tiles per store) to respect the 8 hardware DGE
queues, with w on the SP queue and xT/outputs on the Activation queue.
A short burst of throwaway matmuls during the initial DMA fill lifts
the PE's HAM clock gate before real work arrives.
"""

import numpy as np

import concourse.bass as bass  # noqa: F401  (registers engine classes)
import concourse.mybir as mybir
from concourse import bacc, tile
from concourse.bass_utils import run_bass_kernel_spmd

dt = mybir.dt

BATCH, N_IN, N_OUT = 8192, 4096, 4096
NCORES = 8
MB = BATCH // NCORES          # 1024 batch rows per core
P = 128
KT = N_IN // P                # 32 contraction tiles
MT = MB // P                  # 8 output-row tiles per core
NBLK = 512                    # matmul free dim (one PSUM bank)
NB = N_OUT // NBLK            # 8 output-column blocks
KB = 4                        # k-tiles per k-block (per w DMA)
NKB = KT // KB                # 8 k-blocks
WARMUP_MM = 10

_cache = {}


def _build():
    nc = bacc.Bacc("TRN2", target_bir_lowering=False, debug=False)
    xT = nc.dram_tensor("xT", [N_IN, MB], dt.float32r, kind="ExternalInput")
    w = nc.dram_tensor("w", [N_IN, N_OUT], dt.float32r, kind="ExternalInput")
    b = nc.dram_tensor("b", [1, N_OUT], dt.float32, kind="ExternalInput")
    y = nc.dram_tensor("y", [MB, N_OUT], dt.float32, kind="ExternalOutput")

    xT_r = xT.rearrange("(kt p) m -> p kt m", p=P)    # [128, 32, 1024]
    w_r = w.rearrange("(kt p) n -> p kt n", p=P)      # [128, 32, 4096]
    y_r = y.rearrange("(mt p) n -> p mt n", p=P)      # [128, 8, 4096]

    with tile.TileContext(nc) as tc:
        with (
            tc.tile_pool(name="xtp", bufs=1) as xtp,
            tc.tile_pool(name="wp", bufs=6) as wp,
            tc.tile_pool(name="bp", bufs=1) as bp,
            tc.tile_pool(name="op", bufs=3) as op,
            tc.tile_pool(name="ps", bufs=1, space="PSUM") as ps,
        ):
            # w k-block DMA, 4 k-tiles per transfer on the SP queue.
            # Returns the block as a list of per-k-tile [128, 512] views.
            def w_dma(nb, kb):
                wt = wp.tile([P, KB, NBLK], dt.float32r, name="wt")
                nc.sync.dma_start(
                    wt[:],
                    w_r[:, kb * KB:(kb + 1) * KB, nb * NBLK:(nb + 1) * NBLK],
                )
                return [wt[:, kk, :] for kk in range(KB)]

            xts = xtp.tile([P, KT, MB], dt.float32r, name="xts")

            def xt_dma(kb):
                nc.scalar.dma_start(
                    xts[:, kb * KB:(kb + 1) * KB, :],
                    xT_r[:, kb * KB:(kb + 1) * KB, :],
                )

            # HAM warmup: throwaway matmuls on a zeroed tile while the
            # first DMAs are in flight, so real matmuls start at 2.4 GHz.
            warm = bp.tile([P, 256], dt.float32, name="warm")
            nc.gpsimd.memset(warm[:], 0.0)
            wpsums = [
                ps.tile([P, NBLK], dt.float32, name=f"ps{i}") for i in range(MT)
            ]
            for i in range(WARMUP_MM):
                nc.tensor.matmul(
                    wpsums[i % MT][:, :256], warm[:, :P], warm[:],
                    start=True, stop=True,
                )

            # Startup DMAs in consumption order: the 8 hardware DGE queues
            # are assigned round-robin in emission order and each is FIFO,
            # so a soon-needed transfer must not sit behind a later-needed
            # one at a queue head. The first k-block's transfers are split
            # per k-tile so the first matmul's data lands in ~3 us instead
            # of waiting on multi-MB blocks.
            wts0 = [None] * NKB
            wts0[0] = w_dma(0, 0)
            xt_dma(0)
            wts0[1] = w_dma(0, 1)
            wts0[2] = w_dma(0, 2)
            xt_dma(1)
            wts0[3] = w_dma(0, 3)
            xt_dma(2)
            wts0[4] = w_dma(0, 4)
            xt_dma(3)
            wts0[5] = w_dma(0, 5)
            xt_dma(4)
            wts0[6] = w_dma(0, 6)
            xt_dma(5)
            wts0[7] = w_dma(0, 7)
            for kb in range(6, NKB):
                xt_dma(kb)

            # Bias: DMA the row into partition 0 of bias_sb, then broadcast
            # in place. Emitted after the warmup/startup DMAs — it rides the
            # slow gpsimd queue and is only needed at the first eviction
            # (~95 us in).
            bias_sb = bp.tile([P, N_OUT], dt.float32, name="bias_sb")
            nc.scalar.dma_start(bias_sb[0:1, :], b[:, :])
            nc.gpsimd.partition_broadcast(bias_sb[:], bias_sb[0:1, :])

            for nb in range(NB):
                psums = [
                    ps.tile([P, NBLK], dt.float32, name=f"ps{m}")
                    for m in range(MT)
                ]
                ot = None
                for kb in range(NKB):
                    if nb == 0:
                        wts = wts0[kb]
                    else:
                        wts = w_dma(nb, kb)
                    for m in range(MT):
                        for kk in range(KB):
                            k = kb * KB + kk
                            nc.tensor.matmul(
                                psums[m][:],
                                xts[:, k, m * P:(m + 1) * P],
                                wts[kk],
                                start=(k == 0),
                                stop=(k == KT - 1),
                            )
                        if kb == NKB - 1:
                            if m % 2 == 0:
                                ot = op.tile([P, 2, NBLK], dt.float32, name="ot")
                            nc.vector.tensor_add(
                                ot[:, m % 2, :],
                                psums[m][:],
                                bias_sb[:, nb * NBLK:(nb + 1) * NBLK],
                            )
                            if m % 2 == 1:
                                nc.scalar.dma_start(
                                    y_r[:, m - 1:m + 1, nb * NBLK:(nb + 1) * NBLK],
                                    ot[:],
                                )
    nc.compile()
    return nc


def kernel(x, w, b, _trace=False, _trace_kwargs=None):
    if "nc" not in _cache:
        _cache["nc"] = _build()
    nc = _cache["nc"]

    b2 = np.ascontiguousarray(np.asarray(b, dtype=np.float32).reshape(1, N_OUT))
    w2 = np.ascontiguousarray(np.asarray(w, dtype=np.float32))
    in_maps = []
    for c in range(NCORES):
        xs = np.ascontiguousarray(x[c * MB:(c + 1) * MB].T.astype(np.float32))
        in_maps.append({"xT": xs, "w": w2, "b": b2})

    res = run_bass_kernel_spmd(
        nc,
        in_maps,
        core_ids=list(range(NCORES)),
        trace=_trace,
        **(_trace_kwargs or {}),
    )
    out = np.concatenate([res.results[c]["y"] for c in range(NCORES)], axis=0)
    if _trace:
        return out, res
    return out
